# revision 71
# baseline (speedup 1.0000x reference)
"""Trainium2 Bass kernel for nn_D6BPixelMotifBranch (pooling / memory-bound).

Data-parallel over batch across 8 NeuronCores.

Stage 1 (pooling pass over h): h is uploaded as bf16 in a pixel-paired
[1152, 128] view per batch. Each batch is read twice by DMA: once through
the XBAR transpose engine (dma_start_transpose) to get hT [128(par,d), n/2]
for the slot-logits matmul, and once naturally for the pooling contraction.
No PE transposes, no PSUM->SBUF copies. Softmax over slots runs 8 batches
per pass in [n/2-partition, (g,par,b,k)-free] layout.

Stage 2 (per-chunk transformer block + class attention) is batched 8
batches per 128-partition chunk, emitted interleaved with stage-1 groups.
The Act engine only ever uses {exp, tanh, copy, identity} (one table, no
reloads): gelu is tanh-approx, layernorm rsqrt is a quake-style bit hack +
Newton steps on DVE, and the attention batch-block mask is folded into the
score matmul as a rank-9 accumulate.
"""

import sys
for _p in ("/opt/trn_rl_repo", "/root/.axon_site/_ro/trn_rl_repo"):
    if _p not in sys.path:
        sys.path.append(_p)

import numpy as np
import ml_dtypes

import concourse.bacc as bacc
import concourse.tile as tile
from concourse import mybir

BF16 = mybir.dt.bfloat16
F32 = mybir.dt.float32
I32 = mybir.dt.int32
AF = mybir.ActivationFunctionType
ALU = mybir.AluOpType

# problem dims
B, N, D, K, C, NH, HGT, WID = 256, 2304, 64, 16, 7, 4, 48, 48
DH = D // NH
FF = 4 * D
NCORES = 8

QUAKE_MAGIC = 0x5F3759DF

_bf = lambda x: np.ascontiguousarray(x.astype(ml_dtypes.bfloat16))
_f32 = lambda x: np.ascontiguousarray(x.astype(np.float32))


def _positions(n):
    ys = np.linspace(0.0, 1.0, HGT, dtype=np.float64)
    xs = np.linspace(0.0, 1.0, WID, dtype=np.float64)
    yy, xx = np.meshgrid(ys, xs, indexing="ij")
    pos = np.stack([xx.reshape(-1), yy.reshape(-1)], axis=-1)  # [N,2]
    return pos[:n].astype(np.float64)


def host_prep(params, n=N):
    """Precompute all constant tensors in upload-ready layouts."""
    p = {k: np.asarray(v, dtype=np.float64) for k, v in params.items()}
    scale = 1.0 / np.sqrt(np.float64(D))
    q = p["part_queries"]
    q = q / np.maximum(np.linalg.norm(q, axis=-1, keepdims=True), 1e-6)

    out = {}
    flags = {}

    # slot logits: for parity-paired hT rows (64*par + d), cols (16*par + k)
    AT = (p["Wk_pix"] @ q.T) * scale                     # [64, 16]
    c0 = (p["bk_pix"] @ q.T) * scale                     # [16]
    ATpair = np.zeros((128, 32))
    ATpair[0:64, 0:16] = AT
    ATpair[64:128, 16:32] = AT
    out["ATpair"] = _bf(ATpair)
    flags["c0"] = bool(np.any(c0 != 0.0))
    out["expc0_b"] = _f32(np.tile(np.exp(c0)[None, :], (128, 1)))  # [128,16]

    # positions in paired layout: posE6[p, t, 3*par + c], c in (x, y, 1)
    # pair index p_idx = NS*p + t (partition-contiguous DRAM layout)
    NS = n // 256
    pos = _positions(n)
    posE6 = np.zeros((128, NS, 6))
    for t in range(NS):
        for par in range(2):
            idx = 2 * (NS * np.arange(128) + t) + par
            posE6[:, t, 3 * par + 0] = pos[idx, 0]
            posE6[:, t, 3 * par + 1] = pos[idx, 1]
            posE6[:, t, 3 * par + 2] = 1.0
    out["posE6"] = _bf(posE6)

    # post-pool projection of raw pooled h
    out["WvpixR"] = _bf(np.vstack([p["Wv_pix"], p["Wv_pix"]]))      # [128,64]
    bvb2 = p["bv_pix"] + p["pos_b2"]
    flags["bvb2"] = bool(np.any(bvb2 != 0.0))
    out["bvb2_b"] = _f32(np.tile(bvb2[None, :], (128, 1)))

    # pos mlp
    out["w1x_b"] = _f32(np.tile(p["pos_w1"][0][None, :], (128, 1)))
    out["w1y_b"] = _f32(np.tile(p["pos_w1"][1][None, :], (128, 1)))
    flags["posb1"] = bool(np.any(p["pos_b1"] != 0.0))
    out["posb1_b"] = _f32(np.tile(p["pos_b1"][None, :], (128, 1)))
    out["posw2R"] = _bf(np.vstack([p["pos_w2"], p["pos_w2"]]))

    # attention qkv with head padding: head h at partitions 32h..32h+16
    attn_scale = 1.0 / np.sqrt(np.float64(DH))
    Wq = p["Wqkv"][:, 0:D] * attn_scale
    Wk = p["Wqkv"][:, D:2 * D]
    Wv = p["Wqkv"][:, 2 * D:3 * D]
    bq = p["bqkv"][0:D] * attn_scale
    bk = p["bqkv"][D:2 * D]
    bv = p["bqkv"][2 * D:3 * D]
    Wq_pad = np.zeros((D, 128))
    Wk_pad = np.zeros((D, 128))
    bq_pad = np.zeros(128)
    bk_pad = np.zeros(128)
    for h in range(NH):
        Wq_pad[:, 32 * h:32 * h + DH] = Wq[:, DH * h:DH * (h + 1)]
        Wk_pad[:, 32 * h:32 * h + DH] = Wk[:, DH * h:DH * (h + 1)]
        bq_pad[32 * h:32 * h + DH] = bq[DH * h:DH * (h + 1)]
        bk_pad[32 * h:32 * h + DH] = bk[DH * h:DH * (h + 1)]
    out["WqR"] = _bf(np.vstack([Wq_pad, Wq_pad]))        # [128, 128]
    out["WkR"] = _bf(np.vstack([Wk_pad, Wk_pad]))
    out["bq_pad"] = _f32(bq_pad[:, None])                # [128, 1]
    out["bk_pad"] = _f32(bk_pad[:, None])
    out["WvaR"] = _bf(np.vstack([Wv, Wv]))               # [128, 64]
    flags["bv_attn"] = bool(np.any(bv != 0.0))
    out["bva_b"] = _f32(np.tile(bv[None, :], (128, 1)))

    out["WoR"] = _bf(np.vstack([p["Wo"], p["Wo"]]))
    flags["bo"] = bool(np.any(p["bo"] != 0.0))
    out["bo_b"] = _f32(np.tile(p["bo"][None, :], (128, 1)))

    flags["ln1"] = bool(np.any(p["ln1_g"] != 1.0) or np.any(p["ln1_b"] != 0.0))
    out["ln1g_b"] = _f32(np.tile(p["ln1_g"][None, :], (128, 1)))
    out["ln1b_b"] = _f32(np.tile(p["ln1_b"][None, :], (128, 1)))
    flags["ln2"] = bool(np.any(p["ln2_g"] != 1.0) or np.any(p["ln2_b"] != 0.0))
    out["ln2g_b"] = _f32(np.tile(p["ln2_g"][None, :], (128, 1)))
    out["ln2b_b"] = _f32(np.tile(p["ln2_b"][None, :], (128, 1)))

    # ffn
    W1 = p["ffn_w1"]                                     # [64, 256]
    out["W1aR"] = _bf(np.vstack([W1[:, 0:128], W1[:, 0:128]]))   # [128,128]
    out["W1bR"] = _bf(np.vstack([W1[:, 128:256], W1[:, 128:256]]))
    out["b1a"] = _f32(p["ffn_b1"][0:128, None])          # [128, 1]
    out["b1b"] = _f32(p["ffn_b1"][128:256, None])
    out["W2a"] = _bf(p["ffn_w2"][0:128, :])              # [128, 64]
    out["W2b"] = _bf(p["ffn_w2"][128:256, :])
    flags["b2ffn"] = bool(np.any(p["ffn_b2"] != 0.0))
    out["b2f_b"] = _f32(np.tile(p["ffn_b2"][None, :], (128, 1)))

    # class cross attention
    out["WkclsR"] = _bf(np.vstack([p["Wk_cls"], p["Wk_cls"]]))   # [128, 64]
    out["bkcls"] = _f32(np.concatenate([p["bk_cls"], np.zeros(64)])[:, None])  # [128,1]
    out["CqT"] = _bf((p["class_queries"] * scale).T)     # [64, 7]
    wvec = p["Wv_cls"] @ p["Wh"]                         # [64, 1]
    out["wvecR"] = _bf(np.vstack([wvec, wvec]))          # [128, 1]
    flags["cvh"] = float(p["bv_cls"] @ p["Wh"][:, 0])
    flags["bh"] = float(p["bh"][0])

    out["ident"] = _bf(np.eye(128, dtype=np.float32))    # [128, 128]
    # rank-9 batch-block attention mask: B[i,j] = 0 if i//16==j//16 else -M
    M = 29952.0  # exactly representable in bf16
    U = np.zeros((9, 128))
    V = np.zeros((9, 128))
    U[0, :] = 1.0
    V[0, :] = -M
    for c in range(8):
        U[1 + c, 16 * c:16 * (c + 1)] = 1.0
        V[1 + c, 16 * c:16 * (c + 1)] = M
    # replicated at each 32-row block so the mask matmul can share the
    # score matmul's tile_position (mixing tile configs inside one PSUM
    # accumulation group crashes the PE)
    U4 = np.zeros((128, 128))
    V4 = np.zeros((128, 128))
    for hh in range(4):
        U4[32 * hh:32 * hh + 9, :] = U
        V4[32 * hh:32 * hh + 9, :] = V
    out["Umask"] = _bf(U4)
    out["Vmask"] = _bf(V4)
    # baseline block-diag mask (for the oldattn debug path)
    bd = np.zeros((128, 32), np.float32)
    for pp in range(128):
        half = (pp % 32) // 16
        bd[pp, 16 * half:16 * (half + 1)] = 1.0
    out["bdiag"] = _bf(bd)
    return out, flags


def build(nbatch, n, use_mask, flags):
    """Build the per-core Bass program. nbatch: batches per core (mult of 8)."""
    assert nbatch % 8 == 0 and n % 256 == 0
    NP = n // 2          # pixel pairs per batch
    NS = NP // 128       # 128-pair slices per batch
    NG = nbatch // 8     # groups of 8 batches (== stage-2 chunks)

    nc = bacc.Bacc("TRN2", debug=False)

    hx = nc.dram_tensor("hx", [nbatch, NP, 128], BF16, kind="ExternalInput")
    hxT = nc.dram_tensor("hxT", [nbatch, 128, NP], BF16, kind="ExternalInput")
    if use_mask:
        mk_d = nc.dram_tensor("mask", [nbatch, n], I32, kind="ExternalInput")
    cst_names = [
        ("ATpair", [128, 32], BF16), ("posE6", [128, NS, 6], BF16),
        ("WvpixR", [128, 64], BF16), ("w1x_b", [128, 64], F32),
        ("w1y_b", [128, 64], F32), ("posw2R", [128, 64], BF16),
        ("WqR", [128, 128], BF16), ("WkR", [128, 128], BF16),
        ("bq_pad", [128, 1], F32), ("bk_pad", [128, 1], F32),
        ("WvaR", [128, 64], BF16), ("WoR", [128, 64], BF16),
        ("W1aR", [128, 128], BF16), ("W1bR", [128, 128], BF16),
        ("b1a", [128, 1], F32), ("b1b", [128, 1], F32),
        ("W2a", [128, 64], BF16), ("W2b", [128, 64], BF16),
        ("WkclsR", [128, 64], BF16), ("bkcls", [128, 1], F32),
        ("CqT", [64, C], BF16), ("wvecR", [128, 1], BF16),
        ("ident", [128, 128], BF16),
        ("Umask", [128, 128], BF16), ("Vmask", [128, 128], BF16),
        ("bdiag", [128, 32], BF16),
    ]
    opt_csts = {
        "c0": [("expc0_b", [128, K], F32)],
        "bvb2": [("bvb2_b", [128, 64], F32)],
        "posb1": [("posb1_b", [128, 64], F32)],
        "bv_attn": [("bva_b", [128, 64], F32)],
        "bo": [("bo_b", [128, 64], F32)],
        "ln1": [("ln1g_b", [128, 64], F32), ("ln1b_b", [128, 64], F32)],
        "ln2": [("ln2g_b", [128, 64], F32), ("ln2b_b", [128, 64], F32)],
        "b2ffn": [("b2f_b", [128, 64], F32)],
    }
    for fl, items in opt_csts.items():
        if flags[fl]:
            cst_names.extend(items)
    cst_drams = {nm: nc.dram_tensor(nm, sh, dt, kind="ExternalInput")
                 for nm, sh, dt in cst_names}
    nc._cst_names = [nm for nm, _, _ in cst_names]
    out_d = nc.dram_tensor("out", [nbatch, C], F32, kind="ExternalOutput")

    with tile.TileContext(nc) as tc:
        _build_body(nc, tc, hx, hxT, mk_d if use_mask else None, cst_drams,
                    out_d,
                    nbatch, n, NP, NS, NG, flags)
    nc.compile()
    return nc


def _build_body(nc, tc, hx, hxT, mk_d, cst_drams, out_d, nbatch, n, NP, NS,
                NG, flags):
    from contextlib import ExitStack
    ctx = ExitStack()
    with ctx:
        cst = ctx.enter_context(tc.tile_pool(name="cst", bufs=1))
        # ---- constants into SBUF ----
        sb = {}
        cq = [nc.gpsimd, nc.sync, nc.scalar]
        for ci, (nm, dram) in enumerate(cst_drams.items()):
            t = cst.tile(list(dram.shape), dram.dtype, name=f"c_{nm}")
            cq[ci % 3].dma_start(out=t, in_=dram.ap())
            sb[nm] = t
        ones_bf = cst.tile([128, 1], BF16)
        nc.vector.memset(ones_bf, 1.0)
        ones7 = cst.tile([1, C], BF16)
        nc.vector.memset(ones7, 1.0)
        magic_t = cst.tile([128, 1], I32)
        nc.vector.memset(magic_t, QUAKE_MAGIC)
        # pooled stats per chunk: [128 (8b x 16k), 64 F | 2 Ce | 1 S]
        G = [cst.tile([128, 67], F32, name=f"G{c}") for c in range(NG)]
        OUT_sb = cst.tile([C, nbatch], F32)

        # ---- pools ----
        natp = ctx.enter_context(tc.tile_pool(name="natp", bufs=2))
        htp = ctx.enter_context(tc.tile_pool(name="htp", bufs=2))
        smp = ctx.enter_context(tc.tile_pool(name="smp", bufs=3))
        lgp = ctx.enter_context(tc.tile_pool(name="lgp", bufs=2, space="PSUM"))
        ppp = ctx.enter_context(tc.tile_pool(name="ppp", bufs=2, space="PSUM"))

        s2 = ctx.enter_context(tc.tile_pool(name="s2", bufs=2))
        s2p = ctx.enter_context(tc.tile_pool(name="s2p", bufs=2, space="PSUM"))
        e2p = ctx.enter_context(tc.tile_pool(name="e2p", bufs=4))

        # persistent stage-2 tensors
        qT_sb = cst.tile([128, 128 * NG], BF16)
        kT_sb = cst.tile([128, 128 * NG], BF16)
        KclsT_sb = cst.tile([64, 128 * NG], BF16)
        wv_sb = cst.tile([128, NG], BF16)
        V_sb = cst.tile([128, 64 * NG], BF16)
        NXP = (NG + 1) // 2
        Xpair = [cst.tile([128, 128], BF16, name=f"Xp{i}") for i in range(NXP)]
        Pbarp = [cst.tile([128, 128], BF16, name=f"Pb{i}") for i in range(NXP)]
        h1gp = [cst.tile([128, 128], BF16, name=f"h1g{i}") for i in range(NXP)]
        saNp = [cst.tile([128, 128], BF16, name=f"saN{i}") for i in range(NXP)]
        x1p_t = [cst.tile([128, 128], BF16, name=f"x1p{i}") for i in range(NXP)]
        x2p_t = [cst.tile([128, 128], BF16, name=f"x2p{i}") for i in range(NXP)]
        if NG % 2 == 1:
            for tl in (Xpair, Pbarp, h1gp, saNp, x1p_t, x2p_t):
                nc.vector.memset(tl[NXP - 1], 0.0)

        # ================= stage 1: pooling pass over h =================
        def load_group(g):
            """Issue the DMAs for group g; returns (nat, hT) tiles. Both
            layouts are host-prepared so every DMA is one contiguous
            2304B descriptor per partition (full DMA bandwidth).
            nat[p, j, t, c] = hx[b][NS*p + t, c]; hT[c, j, rr] = hxT[b]."""
            b0 = 8 * g
            nat = natp.tile([128, 8, NS, 128], BF16, tag="nat", name="nat")
            hT = htp.tile([128, 8, NP], BF16, tag="hT", name="hT")
            for j in range(8):
                nc.scalar.dma_start(
                    out=nat[:, j],
                    in_=hx.ap()[b0 + j].rearrange("(p t) c -> p t c", p=128))
                nc.sync.dma_start(out=hT[:, j, :], in_=hxT.ap()[b0 + j])
            return nat, hT

        def stage1_group(g, loaded):
            nat, hT = loaded
            b0 = 8 * g
            hTv = hT.rearrange("a j (p t) -> a j p t", t=NS)
            P = [ppp.tile([128, 512], F32, tag="P", bufs=2, name=f"P{gg}")
                 for gg in range(2)]
            Ppos = [ppp.tile([128, 6], F32, tag=f"Ppos{gg}", bufs=1,
                             name=f"Ppos{gg}") for gg in range(2)]
            for s in range(NS):
                E8 = smp.tile([128, 256], BF16, tag="E8")
                if "nologit" in ABLATE:
                    nc.vector.memset(E8, 1.0)
                else:
                    LG = lgp.tile([128, 256], F32, tag="lg")
                    # col layout: 128*gg + 32*bb + 16*par + k
                    for j in range(8):
                        nc.tensor.matmul(LG[:, 32 * j:32 * (j + 1)],
                                         hTv[:, j, :, s],
                                         sb["ATpair"], start=True, stop=True)
                    nc.scalar.activation(out=E8, in_=LG, func=AF.Exp)
                if flags["c0"]:
                    import concourse.bass as bass
                    ec = sb["expc0_b"]
                    ecb = bass.AP(tensor=ec.tensor, offset=ec.offset,
                                  ap=[ec.ap[0], [0, 16], ec.ap[1]])
                    nc.vector.tensor_tensor(
                        out=E8.rearrange("p (t k) -> p t k", k=K),
                        in0=E8.rearrange("p (t k) -> p t k", k=K),
                        in1=ecb, op=ALU.mult)
                sig = smp.tile([128, 16], F32, tag="sig")
                nc.vector.tensor_reduce(
                    out=sig, in_=E8.rearrange("p (t k) -> p t k", k=K),
                    op=ALU.add, axis=mybir.AxisListType.X)
                rsig = smp.tile([128, 16], F32, tag="rsig")
                nc.vector.reciprocal(out=rsig, in_=sig)
                if mk_d is not None:
                    mk8 = smp.tile([128, 2, 4, 2], I32, tag="mk8")
                    for gg in range(2):
                        nc.sync.dma_start(
                            out=mk8[:, gg],
                            in_=mk_d.ap()[b0 + 4 * gg:b0 + 4 * (gg + 1)]
                            .rearrange("b (p t q) -> p t b q", p=128,
                                       q=2)[:, s])
                    mkf = smp.tile([128, 16], F32, tag="mkf")
                    nc.vector.tensor_copy(
                        out=mkf.rearrange("p (g b q) -> p g b q", g=2, b=4),
                        in_=mk8)
                    nc.vector.tensor_tensor(out=rsig, in0=rsig, in1=mkf,
                                            op=ALU.mult)
                pm8 = smp.tile([128, 256], BF16, tag="pm8")
                import concourse.bass as bass
                rb = bass.AP(tensor=rsig.tensor, offset=rsig.offset,
                             ap=[rsig.ap[0], rsig.ap[1], [0, K]])
                nc.vector.tensor_tensor(
                    out=pm8.rearrange("p (t k) -> p t k", k=K),
                    in0=E8.rearrange("p (t k) -> p t k", k=K),
                    in1=rb, op=ALU.mult)
                if "nopool" not in ABLATE:
                    for gg in range(2):
                        nc.tensor.matmul(P[gg][:, 0:512],
                                         pm8[:, 128 * gg:128 * (gg + 1)],
                                         nat[:, 4 * gg:4 * (gg + 1), s, :],
                                         start=(s == 0), stop=(s == NS - 1))
                        nc.tensor.matmul(Ppos[gg][:, 0:6],
                                         pm8[:, 128 * gg:128 * (gg + 1)],
                                         sb["posE6"][:, s, :],
                                         start=(s == 0), stop=(s == NS - 1))
            # extract batch-diagonal blocks: G row = 16*j + k, j = 4*gg + bb.
            # P[gg] rows: 32*bb + 16*par + k; cols: 128*bb + 64*par + d.
            # Small DMAs (no partition-alignment limits) on the idle Pool
            # engine: par0 diag -> G, par1 diag -> Gp1, then one DVE add.
            # DMA cannot read PSUM, so stage P/Ppos into SBUF first.
            if "nopool" in ABLATE or "noext" in ABLATE:
                nc.vector.memset(G[g], 1.0)
                return
            Gp1 = smp.tile([128, 67], F32, tag="Gp1")
            Psb = smp.tile([128, 2, 512], F32, tag="Psb")
            Psb2 = smp.tile([128, 2, 6], F32, tag="Psb2")
            nc.scalar.copy(out=Psb[:, 0], in_=P[0])
            nc.vector.tensor_copy(out=Psb[:, 1], in_=P[1])
            nc.scalar.copy(out=Psb2[:, 0], in_=Ppos[0])
            nc.vector.tensor_copy(out=Psb2[:, 1], in_=Ppos[1])
            eq = [nc.gpsimd, nc.sync, nc.sync, nc.gpsimd]
            for gg in range(2):
                for bb in range(4):
                    r0 = 16 * (4 * gg + bb)
                    eq[0].dma_start(
                        out=G[g][r0:r0 + 16, 0:64],
                        in_=Psb[32 * bb:32 * bb + 16, gg,
                                128 * bb:128 * bb + 64])
                    eq[1].dma_start(
                        out=Gp1[r0:r0 + 16, 0:64],
                        in_=Psb[32 * bb + 16:32 * bb + 32, gg,
                                128 * bb + 64:128 * bb + 128])
                    eq[2].dma_start(
                        out=G[g][r0:r0 + 16, 64:67],
                        in_=Psb2[32 * bb:32 * bb + 16, gg, 0:3])
                    eq[3].dma_start(
                        out=Gp1[r0:r0 + 16, 64:67],
                        in_=Psb2[32 * bb + 16:32 * bb + 32, gg, 3:6])
            nc.vector.tensor_tensor(out=G[g], in0=G[g], in1=Gp1, op=ALU.add)

        # ================= stage 2 helpers =================
        def xbar(dst, src):
            # PE-based [128,128] transpose (kept on PE: lowest latency)
            ptx = s2p.tile([128, 128], F32, tag="ps", name="ptx")
            nc.tensor.matmul(ptx, src, sb["ident"], start=True, stop=True)
            nc.scalar.copy(out=dst, in_=ptx)

        def gelu(dst, src, bias=None):
            """dst = gelu(src + bias), tanh approximation (keeps the Act
            engine on the exp/tanh table -- no table reloads)."""
            if "hwgelu" in ABLATE:
                nc.scalar.activation(out=dst, in_=src, func=AF.Gelu,
                                     bias=bias if bias is not None else 0.0)
                return
            P_, Fr = src.shape[0], src.shape[1]
            x = s2.tile([128, 128], F32, tag="gx", name="gx")[:P_, :Fr]
            if bias is not None:
                nc.scalar.activation(out=x, in_=src, func=AF.Identity,
                                     bias=bias)
            else:
                nc.scalar.copy(out=x, in_=src)
            t = s2.tile([128, 128], F32, tag="gt2", name="gt2")[:P_, :Fr]
            nc.vector.tensor_tensor(out=t, in0=x, in1=x, op=ALU.mult)
            w = s2.tile([128, 128], F32, tag="gw", name="gw")[:P_, :Fr]
            cst_a = 0.7978845608028654
            nc.vector.tensor_scalar(w, t, cst_a * 0.044715, cst_a,
                                    op0=ALU.mult, op1=ALU.add)
            v = s2.tile([128, 128], F32, tag="gv", name="gv")[:P_, :Fr]
            nc.vector.tensor_tensor(out=v, in0=x, in1=w, op=ALU.mult)
            sT = s2.tile([128, 128], F32, tag="gs", name="gs")[:P_, :Fr]
            nc.scalar.activation(out=sT, in_=v, func=AF.Tanh)
            z = s2.tile([128, 128], F32, tag="gz", name="gz")[:P_, :Fr]
            nc.vector.tensor_scalar(z, sT, 1.0, 0.5, op0=ALU.add,
                                    op1=ALU.mult)
            nc.vector.tensor_tensor(out=dst, in0=z, in1=x, op=ALU.mult)

        def rsqrt(dst, src):
            """dst = 1/sqrt(src) via quake bit hack + 2 Newton steps (DVE
            only -- avoids the Act Sqrt table). src/dst: [128, 1] f32."""
            if "actsqrt" in ABLATE:
                sd = s2.tile([128, 1], F32, tag="q_sd")
                nc.scalar.activation(out=sd, in_=src, func=AF.Sqrt)
                nc.vector.reciprocal(out=dst, in_=sd)
                return
            ih = s2.tile([128, 1], I32, tag="q_ih")
            nc.vector.tensor_scalar(ih, src.bitcast(I32), 1, None,
                                    op0=ALU.logical_shift_right)
            r = s2.tile([128, 1], F32, tag="q_r")
            nc.vector.tensor_tensor(out=r.bitcast(I32), in0=magic_t, in1=ih,
                                    op=ALU.subtract)
            for _ in range(2):
                r2 = s2.tile([128, 1], F32, tag="q_r2")
                nc.vector.tensor_tensor(out=r2, in0=r, in1=r, op=ALU.mult)
                vr2 = s2.tile([128, 1], F32, tag="q_vr2")
                nc.vector.tensor_tensor(out=vr2, in0=src, in1=r2,
                                        op=ALU.mult)
                u = s2.tile([128, 1], F32, tag="q_u")
                nc.vector.tensor_scalar(u, vr2, -0.5, 1.5, op0=ALU.mult,
                                        op1=ALU.add)
                rn = s2.tile([128, 1], F32, tag="q_rn")
                nc.vector.tensor_tensor(out=rn, in0=r, in1=u, op=ALU.mult)
                r = rn
            nc.vector.tensor_copy(out=dst, in_=r)

        def ln(dst, src_f32, gflag, gname, bname):
            st6 = s2.tile([128, 6], F32, tag="ln_st")
            mv = s2.tile([128, 2], F32, tag="ln_mv")
            veps = s2.tile([128, 1], F32, tag="ln_ve")
            rsd = s2.tile([128, 1], F32, tag="ln_rsd")
            nc.vector.bn_stats(out=st6, in_=src_f32)
            nc.vector.bn_aggr(out=mv, in_=st6)
            nc.vector.tensor_scalar_add(veps, mv[:, 1:2], 1e-5)
            rsqrt(rsd, veps)
            if gflag:
                xn = s2.tile([128, 64], F32, tag="ln_xn")
                nc.vector.tensor_scalar(xn, src_f32, mv[:, 0:1], rsd[:, 0:1],
                                        op0=ALU.subtract, op1=ALU.mult)
                xg = s2.tile([128, 64], F32, tag="ln_xg")
                nc.vector.tensor_tensor(out=xg, in0=xn, in1=sb[gname],
                                        op=ALU.mult)
                nc.vector.tensor_tensor(out=dst, in0=xg, in1=sb[bname],
                                        op=ALU.add)
            else:
                nc.vector.tensor_scalar(dst, src_f32, mv[:, 0:1], rsd[:, 0:1],
                                        op0=ALU.subtract, op1=ALU.mult)

        # ================= stage 2: batched per-chunk compute ============
        def stage2_prework(c):
            # per-chunk pre-work; emitted right after chunk c's extraction
            # so it overlaps later stage-1 groups
            tp, o64 = c // 2, 64 * (c % 2)
            Gc = G[c]
            rS = s2.tile([128, 1], F32, tag="rS")
            nc.vector.reciprocal(out=rS, in_=Gc[:, 66:67])
            nc.vector.tensor_scalar_mul(Pbarp[tp][:, o64:o64 + 64],
                                        Gc[:, 0:64], rS[:, 0:1])
            cc2 = s2.tile([128, 2], F32, tag="cc2")
            nc.vector.tensor_scalar_mul(cc2, Gc[:, 64:66], rS[:, 0:1])
            t1 = s2.tile([128, 64], F32, tag="t1")
            nc.vector.tensor_scalar_mul(t1, sb["w1x_b"], cc2[:, 0:1])
            h1 = s2.tile([128, 64], F32, tag="h1")
            nc.vector.scalar_tensor_tensor(out=h1, in0=sb["w1y_b"],
                                           scalar=cc2[:, 1:2], in1=t1,
                                           op0=ALU.mult, op1=ALU.add)
            if flags["posb1"]:
                nc.vector.tensor_tensor(out=h1, in0=h1,
                                        in1=sb["posb1_b"], op=ALU.add)
            gelu(h1gp[tp][:, o64:o64 + 64], h1)

        def stage2_pair(tp):
            # processes chunks 2*tp and 2*tp+1 (each 8 batches)
            chunks = [c for c in (2 * tp, 2 * tp + 1) if c < NG]
            PbarT = s2.tile([128, 128], BF16, tag="PbarT", bufs=2)
            h1gT = s2.tile([128, 128], BF16, tag="h1gT", bufs=2)
            xbar(PbarT, Pbarp[tp])
            xbar(h1gT, h1gp[tp])
            for c2 in chunks:
                o2 = 64 * (c2 % 2)
                X0 = s2p.tile([128, 64], F32, tag="ps")
                nc.tensor.matmul(X0, PbarT[o2:o2 + 64, :],
                                 sb["WvpixR"][o2:o2 + 64, :],
                                 start=True, stop=False)
                nc.tensor.matmul(X0, h1gT[o2:o2 + 64, :],
                                 sb["posw2R"][o2:o2 + 64, :],
                                 start=False, stop=True)
                if flags["bvb2"]:
                    nc.vector.tensor_tensor(
                        out=Xpair[tp][:, o2:o2 + 64], in0=X0,
                        in1=sb["bvb2_b"], op=ALU.add)
                else:
                    nc.scalar.copy(out=Xpair[tp][:, o2:o2 + 64], in_=X0)
            XT = s2.tile([128, 128], BF16, tag="XT", bufs=2)
            xbar(XT, Xpair[tp])
            for c2 in chunks:
                o2 = 64 * (c2 % 2)
                qTp = s2p.tile([128, 128], F32, tag="ps")
                nc.tensor.matmul(qTp, sb["WqR"][o2:o2 + 64, :],
                                 XT[o2:o2 + 64, :], start=True, stop=True)
                nc.scalar.activation(out=qT_sb[:, 128 * c2:128 * (c2 + 1)],
                                     in_=qTp, func=AF.Identity,
                                     bias=sb["bq_pad"][:, 0:1])
                kTp = s2p.tile([128, 128], F32, tag="ps")
                nc.tensor.matmul(kTp, sb["WkR"][o2:o2 + 64, :],
                                 XT[o2:o2 + 64, :], start=True, stop=True)
                nc.scalar.activation(out=kT_sb[:, 128 * c2:128 * (c2 + 1)],
                                     in_=kTp, func=AF.Identity,
                                     bias=sb["bk_pad"][:, 0:1])
                Vp = s2p.tile([128, 64], F32, tag="ps")
                nc.tensor.matmul(Vp, XT[o2:o2 + 64, :],
                                 sb["WvaR"][o2:o2 + 64, :],
                                 start=True, stop=True)
                if flags["bv_attn"]:
                    nc.vector.tensor_tensor(
                        out=V_sb[:, 64 * c2:64 * (c2 + 1)], in0=Vp,
                        in1=sb["bva_b"], op=ALU.add)
                else:
                    nc.scalar.copy(out=V_sb[:, 64 * c2:64 * (c2 + 1)],
                                   in_=Vp)
            # attention: head-outer, chunk-inner emission so PE works on
            # one chunk's scores while the Act engine exps the other's
            if "noattn" in ABLATE:
                nc.vector.memset(saNp[tp], 0.125)
            else:
                for c2 in chunks:
                    o2 = 64 * (c2 % 2)
                    SA = s2p.tile([128, 68], F32, tag="ps", name="SA")
                    for h in range(NH):
                        SC = s2p.tile([128, 128], F32, tag="ps")
                        nc.tensor.matmul(
                            SC,
                            qT_sb[32 * h:32 * h + DH, 128 * c2:128 * (c2 + 1)],
                            kT_sb[32 * h:32 * h + DH, 128 * c2:128 * (c2 + 1)],
                            start=True, stop=False, tile_position=(32 * h, 0))
                        # rank-9 accumulate: 0 on diagonal 16-blocks, -M off
                        nc.tensor.matmul(SC,
                                         sb["Umask"][32 * h:32 * h + 9, :],
                                         sb["Vmask"][32 * h:32 * h + 9, :],
                                         start=False, stop=True,
                                         tile_position=(32 * h, 0))
                        E2 = e2p.tile([128, 128], BF16, tag="E2")
                        nc.scalar.activation(out=E2, in_=SC, func=AF.Exp)
                        nc.tensor.matmul(
                            SA[:, 16 * h:16 * (h + 1)], E2,
                            V_sb[:, 64 * c2 + 16 * h:64 * c2 + 16 * (h + 1)],
                            start=(h == 0), stop=False)
                        nc.tensor.matmul(SA[:, 64 + h:65 + h], E2,
                                         ones_bf[:, 0:1], start=False,
                                         stop=(h == NH - 1))
                    rR = s2.tile([128, 4], F32, tag="rR")
                    nc.vector.reciprocal(out=rR, in_=SA[:, 64:68])
                    for h in range(NH):
                        nc.vector.tensor_scalar_mul(
                            saNp[tp][:, o2 + 16 * h:o2 + 16 * (h + 1)],
                            SA[:, 16 * h:16 * (h + 1)], rR[:, h:h + 1])
            saNT = s2.tile([128, 128], BF16, tag="saNT", bufs=2)
            xbar(saNT, saNp[tp])
            for c2 in chunks:
                o2 = 64 * (c2 % 2)
                x1ps = s2p.tile([128, 64], F32, tag="ps")
                nc.tensor.matmul(x1ps, saNT[o2:o2 + 64, :],
                                 sb["WoR"][o2:o2 + 64, :],
                                 start=True, stop=True)
                x1s = s2.tile([128, 64], F32, tag="x1s")
                nc.vector.tensor_tensor(out=x1s, in0=x1ps,
                                        in1=Xpair[tp][:, o2:o2 + 64],
                                        op=ALU.add)
                if flags["bo"]:
                    nc.vector.tensor_tensor(out=x1s, in0=x1s,
                                            in1=sb["bo_b"], op=ALU.add)
                ln(x1p_t[tp][:, o2:o2 + 64], x1s, flags["ln1"],
                   "ln1g_b", "ln1b_b")
            x1T = s2.tile([128, 128], BF16, tag="x1T", bufs=2)
            xbar(x1T, x1p_t[tp])
            for c2 in chunks:
                o2 = 64 * (c2 % 2)
                # emit the two f1 matmuls back-to-back (own PSUM tag with
                # bufs=4 so both chunks' mms run before the gelus finish)
                f1a = s2p.tile([128, 128], F32, tag="ps", name="f1a")
                nc.tensor.matmul(f1a, sb["W1aR"][o2:o2 + 64, :],
                                 x1T[o2:o2 + 64, :], start=True, stop=True)
                f1b = s2p.tile([128, 128], F32, tag="ps", name="f1b")
                nc.tensor.matmul(f1b, sb["W1bR"][o2:o2 + 64, :],
                                 x1T[o2:o2 + 64, :], start=True, stop=True)
                fga = s2.tile([128, 128], BF16, tag="fga")
                gelu(fga, f1a, bias=sb["b1a"][:, 0:1])
                fgb = s2.tile([128, 128], BF16, tag="fgb")
                gelu(fgb, f1b, bias=sb["b1b"][:, 0:1])
                f2 = s2p.tile([128, 64], F32, tag="ps")
                nc.tensor.matmul(f2, fga, sb["W2a"], start=True,
                                 stop=False)
                nc.tensor.matmul(f2, fgb, sb["W2b"], start=False,
                                 stop=True)
                x2s = s2.tile([128, 64], F32, tag="x2s")
                nc.vector.tensor_tensor(out=x2s, in0=f2,
                                        in1=x1p_t[tp][:, o2:o2 + 64],
                                        op=ALU.add)
                if flags["b2ffn"]:
                    nc.vector.tensor_tensor(out=x2s, in0=x2s,
                                            in1=sb["b2f_b"], op=ALU.add)
                ln(x2p_t[tp][:, o2:o2 + 64], x2s, flags["ln2"],
                   "ln2g_b", "ln2b_b")
            x2T = s2.tile([128, 128], BF16, tag="x2T", bufs=2)
            xbar(x2T, x2p_t[tp])
            for c2 in chunks:
                o2 = 64 * (c2 % 2)
                KTp = s2p.tile([64, 128], F32, tag="ps")
                nc.tensor.matmul(KTp, sb["WkclsR"][o2:o2 + 64, :],
                                 x2T[o2:o2 + 64, :], start=True, stop=True)
                nc.scalar.activation(
                    out=KclsT_sb[:, 128 * c2:128 * (c2 + 1)], in_=KTp,
                    func=AF.Identity, bias=sb["bkcls"][0:64, 0:1])
                wvp = s2p.tile([128, 1], F32, tag="ps")
                nc.tensor.matmul(wvp, x2T[o2:o2 + 64, :],
                                 sb["wvecR"][o2:o2 + 64, :],
                                 start=True, stop=True)
                nc.scalar.activation(out=wv_sb[:, c2:c2 + 1], in_=wvp,
                                     func=AF.Copy, bias=flags["cvh"])
            # class attention, pair-wide: [C, 128*len(chunks)]
            nck = len(chunks)
            c0 = chunks[0]
            W_ = 128 * nck
            SCc = s2p.tile([C, 256], F32, tag="ps", name="scc")[:, 0:W_]
            nc.tensor.matmul(SCc, sb["CqT"],
                             KclsT_sb[:, 128 * c0:128 * c0 + W_],
                             start=True, stop=True)
            E2c = s2.tile([C, 256], F32, tag="E2c", name="E2c")[:, 0:W_]
            nc.scalar.activation(out=E2c, in_=SCc, func=AF.Exp)
            wv7 = s2p.tile([C, 256], F32, tag="ps", name="wv7")[:, 0:W_]
            for ci in range(nck):
                wvTp = s2p.tile([1, 128], F32, tag="ps", name="wvTp")
                nc.tensor.matmul(wvTp, wv_sb[:, c0 + ci:c0 + ci + 1],
                                 sb["ident"], start=True, stop=True)
                wvTs = s2.tile([1, 128], BF16, tag="wvTs", name="wvTs")
                nc.scalar.copy(out=wvTs, in_=wvTp)
                nc.tensor.matmul(wv7[:, 128 * ci:128 * (ci + 1)], ones7,
                                 wvTs, start=True, stop=True)
            prod = s2.tile([C, 256], F32, tag="prod", name="prod")[:, 0:W_]
            nc.vector.tensor_tensor(out=prod, in0=E2c, in1=wv7,
                                    op=ALU.mult)
            num = s2.tile([C, 16], F32, tag="num", name="num")[:, 0:8 * nck]
            nc.vector.reduce_sum(
                out=num, in_=prod.rearrange("c (b k) -> c b k", k=16),
                axis=mybir.AxisListType.X)
            den = s2.tile([C, 16], F32, tag="den", name="den")[:, 0:8 * nck]
            nc.vector.reduce_sum(
                out=den, in_=E2c.rearrange("c (b k) -> c b k", k=16),
                axis=mybir.AxisListType.X)
            rden = s2.tile([C, 16], F32, tag="rden", name="rden")[:, 0:8 * nck]
            nc.vector.reciprocal(out=rden, in_=den)
            oc = s2.tile([C, 16], F32, tag="oc", name="oc")[:, 0:8 * nck]
            nc.vector.tensor_tensor(out=oc, in0=num, in1=rden,
                                    op=ALU.mult)
            nc.vector.tensor_scalar_add(
                OUT_sb[:, 8 * c0:8 * c0 + 8 * nck], oc, flags["bh"])

        # ---- emit: stage-1 groups with stage-2 pairs interleaved ----
        if "nostage2" in ABLATE:
            nc.vector.memset(OUT_sb, 0.0)
        loaded = load_group(0)
        for g in range(NG):
            nxt = load_group(g + 1) if g + 1 < NG else None
            stage1_group(g, loaded)
            loaded = nxt
            if "nostage2" not in ABLATE:
                stage2_prework(g)
                if g % 2 == 1:
                    stage2_pair(g // 2)
        if "nostage2" not in ABLATE and NG % 2 == 1:
            stage2_pair(NG // 2)
        nc.sync.dma_start(out=out_d.ap().rearrange("b c -> c b"), in_=OUT_sb)


_CACHE = {}
TRACE = False          # test harness can set kernel.TRACE = True
LAST_RESULT = None     # BassKernelResults of the last kernel() call
ABLATE = set()         # debug: {"nostage2","nopool","nologit","noxbar","noext"}


def _get_program(nbatch, n, use_mask, flags):
    key = (nbatch, n, use_mask, tuple(sorted(
        (k, v) for k, v in flags.items() if isinstance(v, bool))))
    if key not in _CACHE:
        _CACHE[key] = build(nbatch, n, use_mask, flags)
    return _CACHE[key]


def kernel(**inputs):
    from concourse.bass_utils import run_bass_kernel_spmd

    h_pixel = np.asarray(inputs["h_pixel"], dtype=np.float32)
    node_mask = np.ascontiguousarray(np.asarray(inputs["node_mask"],
                                                dtype=np.int32))
    b, n, d = h_pixel.shape
    params = {k: v for k, v in inputs.items()
              if k not in ("h_pixel", "node_mask")}
    csts, flags = host_prep(params, n=n)
    use_mask = bool(not np.all(node_mask == 1))
    nbatch = b // NCORES
    nc = _get_program(nbatch, n, use_mask, flags)

    hx = np.ascontiguousarray(h_pixel.astype(ml_dtypes.bfloat16)).reshape(
        b, n // 2, 128)
    hxT = np.ascontiguousarray(hx.transpose(0, 2, 1))
    in_maps = []
    for core in range(NCORES):
        m = {"hx": hx[core * nbatch:(core + 1) * nbatch],
             "hxT": hxT[core * nbatch:(core + 1) * nbatch]}
        if use_mask:
            m["mask"] = node_mask[core * nbatch:(core + 1) * nbatch]
        for k in nc._cst_names:
            m[k] = csts[k]
        in_maps.append(m)
    kwargs = {}
    if TRACE:
        kwargs["trace"] = True
    res = run_bass_kernel_spmd(nc, in_maps, core_ids=list(range(NCORES)),
                               **kwargs)
    global LAST_RESULT
    LAST_RESULT = res
    out = np.concatenate([r["out"] for r in res.results], axis=0)
    return out.astype(np.float32)


if __name__ == "__main__":
    import jax
    sys.path.insert(0, "/root/problem")
    import reference
    inputs = {k: np.asarray(v) for k, v in reference.setup_inputs().items()}
    got = kernel(**inputs)
    print("out shape", got.shape)
